# revision 21
# baseline (speedup 1.0000x reference)
"""Trainium2 Bass kernel for nn_CCepLTVFilter.

Pipeline (per core; frequency-sharded across 8 cores, FS=128 freqs each):
  1. conv1d(x, W) + b            -> ccep[o, bt]          (PE, fp16 1-pass)
  2. Yr/Yi = DFT of padded ccep  -> [f, bt]              (PE, lhsT = CF/SF)
  3. mag = 10^(Yr/10) via tanh identity; paired sin/cos via two
     [128,512] ACT Sin ops: cs = [cos|sin], ns = [-sin|cos]
     (sin(x+pi) = -sin(x); single preloaded ACT table #18)
  4. Zr/Zi = 1025-pt DFT of z frames (host-transposed)   (PE, fp16)
  5. P = rn (cos + i sin)(zr + i zi) as three wide DVE ops:
     avdv = cs*[zr|zr], bvcv = ns*[zi|zi], u1u2 = avdv + bvcv,
     P = rn2 * u1u2 written straight into the guard-column layout
  6. OLA fused into the final DFT: out_b[t, 0:HOP] =
     Pr_b^T CO_l + Prs_b^T CO_r + Pi_b^T SO_l + Pis_b^T SO_r (PE)
     where the t-1 circular shift comes from 1-col guard columns in the
     packed P tile [128, 520] = [g|Pr_b0|g|Pr_b1|pad|g|Pi_b0|g|Pi_b1].

All matmul inputs are fp16 (m10 keeps rel err ~7e-3 vs the 2e-2 gate;
fp16 halves both DMA bytes and PE passes). CO/SO carry the Hann window
and a x16 rescale (fp16-normal range); 1/(16*1025) is folded into rn.
Inputs are packed into wide-row DMAs (>=2KB/partition) so HBM
transfers run near peak. Per-core outputs are partial sums of the full
[2,1,32768] output; summed on the host.
"""

import numpy as np

import concourse.bass as bass
import concourse.bacc as bacc
import concourse.mybir as mybir
import concourse.tile as tile
from concourse.bass_utils import run_bass_kernel_spmd
from concourse.instruction_name_ordered_set import InstructionNameOrderedSet

# ---------------- problem dims (hardcoded) ----------------
B, T, D = 2, 128, 80
CCEP = 222
FFT = 1024
HOP = 256
WIN = 2 * HOP            # 512
PAD = (FFT - CCEP) // 2  # 401
M = FFT + 1              # 1025-point transforms
BT = B * T               # 256
NCORES = 8
FS = FFT // NCORES       # 128 frequencies per core
OC = CCEP // 2           # 111 (o-chunk)
LAM = float(np.log(10.0) / 10.0)
COSC = 16.0              # CO/SO rescale; 1/(COSC*M) folded into rn

F32 = mybir.dt.float32
F16 = mybir.dt.float16
PI = float(np.pi)

ACT_TABLE_SIN_TANH = 18  # silu_and_others: covers Copy, Tanh, Sin

TRACE = False            # set by test harness for profiling
LAST_RESULT = None       # BassKernelResults of last run (for test harness)


# ---------------- host-side constants (input independent) ----------------
def _make_constants():
    o = np.arange(CCEP, dtype=np.float64)[:, None]
    f = np.arange(FFT, dtype=np.float64)[None, :]
    qn_idx = np.arange(1, CCEP // 2 + 1, dtype=np.float64)
    qnorm = np.concatenate([qn_idx[::-1], qn_idx])
    ang = 2.0 * np.pi * f * (o + PAD) / FFT
    CF = np.cos(ang) * (LAM / 2.0) / qnorm[:, None]      # [222,1024]
    SF = -np.sin(ang) / qnorm[:, None]

    u = np.arange(WIN, dtype=np.float64)[:, None]
    phi = 2.0 * np.pi * f * (u + FFT // 2) / M
    ZC = np.cos(phi)                                     # [512,1024]
    ZS = np.sin(phi)

    w = np.arange(WIN, dtype=np.float64)[None, :]
    th = 2.0 * np.pi * np.arange(FFT, dtype=np.float64)[:, None] * w / M
    win = 0.5 * (1.0 - np.cos(2.0 * np.pi * np.arange(WIN) / WIN))
    CO = np.cos(th) * win[None, :] * COSC                # [1024,512]
    SO = np.sin(th) * win[None, :] * COSC

    consts = []
    for c in range(NCORES):
        sl = slice(c * FS, (c + 1) * FS)
        # cf/sf [111, 256] each, cols (oc, f) -> packed into spk on host
        cfp = CF[:, sl].reshape(2, OC, FS).transpose(1, 0, 2).reshape(OC, 2 * FS)
        sfp = SF[:, sl].reshape(2, OC, FS).transpose(1, 0, 2).reshape(OC, 2 * FS)
        cp1 = np.zeros((128, 4 * FS), np.float32)
        cp1[0:OC, 0:2 * FS] = cfp
        cp1[0:OC, 2 * FS:4 * FS] = sfp
        # zczs [128, 1024]: zc(u-chunk mc, f) | zs
        zcp = ZC[:, sl].reshape(4, 128, FS).transpose(1, 0, 2).reshape(128, 4 * FS)
        zsp = ZS[:, sl].reshape(4, 128, FS).transpose(1, 0, 2).reshape(128, 4 * FS)
        zczs = np.concatenate([zcp, zsp], axis=1)
        # cp3 [128, 1024] = [co_l | co_r | so_l | so_r]
        cob, sob = CO[sl, :], SO[sl, :]
        cp3 = np.concatenate(
            [cob[:, :HOP], cob[:, HOP:], sob[:, :HOP], sob[:, HOP:]], axis=1)
        consts.append(dict(cp1=cp1.astype(np.float16),
                           zczs=zczs.astype(np.float16),
                           cp3=cp3.astype(np.float16)))
    return consts


_CONSTS = _make_constants()
_NC = None


# ---------------- device program ----------------
def _build_nc():
    nc = bacc.Bacc()
    # spk = spack (xcat+w2, 956 cols) | cp1 (cf+sf, 512 cols): 2936B rows.
    # DMA'd as two slices on the same queue (conv only needs the first).
    sp_e = nc.dram_tensor("spk", [128, 1468], F16, kind="ExternalInput")
    fr_e = nc.dram_tensor("frp", [128, 4 * BT], F16, kind="ExternalInput")
    zz_e = nc.dram_tensor("zczs", [128, 8 * FS], F16, kind="ExternalInput")
    c3_e = nc.dram_tensor("cp3", [128, 8 * FS], F16, kind="ExternalInput")
    out_e = nc.dram_tensor("out", [B, 1, T * HOP], F32, kind="ExternalOutput")

    with tile.TileContext(nc) as tc:
        with tc.tile_pool(name="sb", bufs=1) as sb, \
             tc.tile_pool(name="ps", bufs=1, space="PSUM") as ps:

            # ---- input DMAs; queue = issuing engine ----
            spk = sb.tile([128, 1468], F16, tag="spk", name="spk")
            nc.sync.dma_start(out=spk[:, 0:956], in_=sp_e[:, 0:956])
            nc.sync.dma_start(out=spk[:, 956:1468], in_=sp_e[:, 956:1468])
            frp = sb.tile([128, 4 * BT], F16, tag="frp", name="frp")
            nc.scalar.dma_start(out=frp[:], in_=fr_e[:, :])
            zczs = sb.tile([128, 8 * FS], F16, tag="zczs", name="zczs")
            nc.gpsimd.dma_start(out=zczs[:], in_=zz_e[:, :])
            cp3 = sb.tile([128, 8 * FS], F16, tag="cp3", name="cp3")
            nc.gpsimd.dma_start(out=cp3[:], in_=c3_e[:, :])

            xcatA = spk[0:121, 0:BT]
            xcatB = spk[0:120, BT:2 * BT]
            w2A = spk[0:121, 2 * BT:2 * BT + CCEP]
            w2B = spk[0:120, 2 * BT + CCEP:2 * BT + 2 * CCEP]
            cf = spk[0:OC, 956:956 + 2 * FS]
            sf = spk[0:OC, 956 + 2 * FS:956 + 4 * FS]
            zc = zczs[:, 0:4 * FS]
            zs = zczs[:, 4 * FS:8 * FS]
            co_l = cp3[:, 0:HOP]
            co_r = cp3[:, HOP:2 * HOP]
            so_l = cp3[:, 2 * HOP:3 * HOP]
            so_r = cp3[:, 3 * HOP:4 * HOP]

            # ---- conv: ccep[o, bt] = W2.T @ xcat (bias via ones row) ----
            ccep = []
            for c in range(2):
                pc = ps.tile([OC, BT], F32, tag=f"conv{c}", name=f"conv{c}")
                nc.tensor.matmul(pc[:, :], w2A[:, c * OC:(c + 1) * OC],
                                 xcatA, start=True, stop=False)
                nc.tensor.matmul(pc[:, :], w2B[:, c * OC:(c + 1) * OC],
                                 xcatB, start=False, stop=True)
                cs_ = sb.tile([OC, BT], F16, tag=f"ccep{c}", name=f"ccep{c}")
                # half on scalar, half on vector: halves the copy latency
                nc.scalar.copy(cs_[:, 0:T], pc[:, 0:T])
                nc.vector.tensor_copy(cs_[:, T:BT], pc[:, T:BT])
                ccep.append(cs_)

            # ---- step2: Yr/Yi [f_local, bt] ----
            yr = ps.tile([FS, BT], F32, tag="yr", name="yr")
            yi = ps.tile([FS, BT], F32, tag="yi", name="yi")
            for c in range(2):
                nc.tensor.matmul(yr[:, :], cf[:, c * FS:(c + 1) * FS],
                                 ccep[c][:, :], start=(c == 0), stop=(c == 1))
            for c in range(2):
                yi_mm = nc.tensor.matmul(yi[:, :], sf[:, c * FS:(c + 1) * FS],
                                         ccep[c][:, :],
                                         start=(c == 0), stop=(c == 1))

            # ---- step3 ----
            # ACT: tanh (feeds den/num), then three Sin ops building
            # csns [128, 1024] = [cos | sin | -sin | cos]
            th = sb.tile([FS, BT], F32, tag="th", name="th")
            nc.scalar.activation(th[:, :], yr[:, :],
                                 mybir.ActivationFunctionType.Tanh)
            # wr3 = [yic | yiw | yiwn]: wrapped yi+pi/2, yi, yi+pi
            wr3 = sb.tile([FS, 3 * BT], F32, tag="wr3", name="wr3")
            nc.vector.add_range_wrap(wr3[:, 0:BT], yi[:, :],
                                     PI / 2.0, PI, 2.0 * PI)
            nc.vector.add_range_wrap(wr3[:, BT:2 * BT], yi[:, :],
                                     0.0, PI, 2.0 * PI)
            nc.vector.add_range_wrap(wr3[:, 2 * BT:3 * BT], yi[:, :],
                                     PI, PI, 2.0 * PI)
            # csns [128, 768] = [cos | sin | -sin]; ns is a strided view
            # [-sin | cos] (cols 512:768 then 0:256)
            csns = sb.tile([FS, 3 * BT], F16, tag="csns", name="csns")
            for g in range(3):
                nc.scalar.activation(csns[:, g * BT:(g + 1) * BT],
                                     wr3[:, g * BT:(g + 1) * BT],
                                     mybir.ActivationFunctionType.Sin)
            cs = csns[:, 0:2 * BT]       # [cos | sin]
            csf = csns[:, :]
            ns = bass.AP(csf.tensor, csf.offset + 2 * BT,
                         [csf.ap[0], [-2 * BT, 2], [1, BT]])
            # rn = (1+t)/(1-t) / (M*COSC) ; den/num/rn on GPSIMD frees DVE
            den = sb.tile([FS, BT], F32, tag="den", name="den")
            nc.gpsimd.tensor_scalar(den[:, :], th[:, :], -1.0, 1.0,
                                    mybir.AluOpType.mult, mybir.AluOpType.add)
            s = 1.0 / (M * COSC)
            num = sb.tile([FS, BT], F32, tag="num", name="num")
            nc.gpsimd.tensor_scalar(num[:, :], th[:, :], s, s,
                                    mybir.AluOpType.mult, mybir.AluOpType.add)
            rcp = sb.tile([FS, BT], F32, tag="rcp", name="rcp")
            nc.vector.reciprocal_approx_fast(rcp[:, :], den[:, :])

            # ---- step4: Zr/Zi [f_local, bt] ----
            zr = ps.tile([FS, BT], F32, tag="zr", name="zr")
            zi = ps.tile([FS, BT], F32, tag="zi", name="zi")
            for mc in range(4):
                zr_mm = nc.tensor.matmul(zr[:, :], zc[:, mc * FS:(mc + 1) * FS],
                                         frp[:, mc * BT:(mc + 1) * BT],
                                         start=(mc == 0), stop=(mc == 3))
                if mc == 0:
                    # ordering-only dep: keep the PE queue in conv -> yr/yi
                    # -> zr/zi order (the scheduler's DMA model would
                    # otherwise front-run zr and head-of-line-block conv)
                    s_ = InstructionNameOrderedSet()
                    s_.add(yi_mm.ins.name)
                    zr_mm.ins.add_nosync_dependencies_from(s_)
            for mc in range(4):
                zi_mm = nc.tensor.matmul(zi[:, :], zs[:, mc * FS:(mc + 1) * FS],
                                         frp[:, mc * BT:(mc + 1) * BT],
                                         start=(mc == 0), stop=(mc == 3))
                if mc == 0:
                    s_ = InstructionNameOrderedSet()
                    s_.add(yi_mm.ins.name)
                    zi_mm.ins.add_nosync_dependencies_from(s_)

            # ---- step5 (DVE, wide fused ops) ----
            def rep2(pt):
                full = pt[:, :]
                return bass.AP(full.tensor, full.offset,
                               [full.ap[0], [0, 2], [1, BT]])

            avdv = sb.tile([FS, 2 * BT], F16, tag="avdv", name="avdv")
            nc.vector.tensor_tensor(avdv[:, :], cs[:, :], rep2(zr),
                                    mybir.AluOpType.mult)
            bvcv = sb.tile([FS, 2 * BT], F16, tag="bvcv", name="bvcv")
            nc.vector.tensor_tensor(bvcv[:, :], ns, rep2(zi),
                                    mybir.AluOpType.mult)
            u1u2 = sb.tile([FS, 2 * BT], F16, tag="u1u2", name="u1u2")
            nc.vector.tensor_tensor(u1u2[:, :], avdv[:, :], bvcv[:, :],
                                    mybir.AluOpType.add)
            rn = sb.tile([FS, BT], F16, tag="rn", name="rn")
            nc.gpsimd.tensor_tensor(rn[:, :], num[:, :], rcp[:, :],
                                    mybir.AluOpType.mult)

            # pq [128, 520] = [g|Pr_b0|g|Pr_b1|pad2 | g|Pi_b0|g|Pi_b1|pad2]
            pq = sb.tile([FS, 520], F16, tag="pq", name="pq")
            pqf = pq[:, :]
            pdst = bass.AP(pqf.tensor, pqf.offset + 1,
                           [pqf.ap[0], [260, 2], [129, 2], [1, T]])
            rnf = rn[:, :]
            rn4 = bass.AP(rnf.tensor, rnf.offset,
                          [rnf.ap[0], [0, 2], [T, 2], [1, T]])
            uf = u1u2[:, :]
            u4 = bass.AP(uf.tensor, uf.offset,
                         [uf.ap[0], [2 * T, 2], [T, 2], [1, T]])
            nc.vector.tensor_tensor(pdst, rn4, u4, mybir.AluOpType.mult)
            wdst = bass.AP(pqf.tensor, pqf.offset,
                           [pqf.ap[0], [260, 2], [129, 2], [1, 1]])
            wsrc = bass.AP(pqf.tensor, pqf.offset + T,
                           [pqf.ap[0], [260, 2], [129, 2], [1, 1]])
            nc.scalar.copy(wdst, wsrc)

            # ---- step6 + OLA (fused): per b, out_ob[t, j] =
            #  Pr_b^T co_l + Prs_b^T co_r + Pi_b^T so_l + Pis_b^T so_r
            for bb in range(B):
                opr = 1 + bb * 129
                opi = 261 + bb * 129
                pr_b = pq[:, opr:opr + T]
                prs_b = pq[:, opr - 1:opr - 1 + T]
                pi_b = pq[:, opi:opi + T]
                pis_b = pq[:, opi - 1:opi - 1 + T]
                ob = ps.tile([T, HOP], F32, tag=f"ob{bb}", name=f"ob{bb}")
                # unshifted terms first: the guard-col wrap copy lands
                # while they run
                nc.tensor.matmul(ob[:, :], pr_b, co_l, start=True, stop=False)
                nc.tensor.matmul(ob[:, :], pi_b, so_l, start=False, stop=False)
                nc.tensor.matmul(ob[:, :], prs_b, co_r, start=False, stop=False)
                nc.tensor.matmul(ob[:, :], pis_b, so_r, start=False, stop=True)
                obs = sb.tile([T, HOP], F32, tag=f"obs{bb}", name=f"obs{bb}")
                if bb == 0:
                    nc.scalar.copy(obs[:, :], ob[:, :])
                else:
                    # halve the tail latency: copy halves on both engines
                    nc.vector.tensor_copy(obs[:, 0:T], ob[:, 0:T])
                    nc.scalar.copy(obs[:, T:HOP], ob[:, T:HOP])
                dst = bass.AP(out_e[:, :, :].tensor, bb * T * HOP,
                              [[HOP, T], [1, HOP]])
                eng = nc.sync if bb == 0 else nc.scalar
                eng.dma_start(out=dst, in_=obs[:, :])

    return nc


def _patch_act_table(nc):
    """Pre-place a single ACT table load (table 18 covers Copy+Tanh+Sin)
    instead of the default pass's two loads (exp table then trig table).
    Placed just before the first InstActivation so the scalar engine's
    DMA issues at the top of the block are not delayed by the load."""
    def my_insert():
        for b in nc.main_func.blocks:
            idx = None
            for j, i in enumerate(b.instructions):
                if isinstance(i, mybir.InstActivation):
                    idx = j
                    break
            if idx is None:
                continue
            ld = mybir.InstLoadActFuncSet(
                name=nc.get_next_instruction_name(),
                act_func_set_id=ACT_TABLE_SIN_TANH, ins=[], outs=[])
            ld.engine = mybir.EngineType.Activation
            nc.register_instruction(ld)
            b.instructions.insert(idx, ld)
            return
    nc.insert_act_table_loads = my_insert


def _get_nc():
    global _NC
    if _NC is None:
        _NC = _build_nc()
        _patch_act_table(_NC)
        _NC.finalize()
    return _NC


# ---------------- host orchestration ----------------
def _prep_inputs(x, z, W, b, cp1):
    x = np.ascontiguousarray(np.asarray(x, dtype=np.float32))
    z = np.ascontiguousarray(np.asarray(z, dtype=np.float32))
    W = np.ascontiguousarray(np.asarray(W, dtype=np.float32))
    b = np.ascontiguousarray(np.asarray(b, dtype=np.float32))

    xT = np.ascontiguousarray(x.reshape(BT, D).T)                 # [80, 256]
    xsh = np.zeros((3, D, BT), np.float32)
    xsh[1] = xT
    xv = xT.reshape(D, B, T)
    xsh[0].reshape(D, B, T)[:, :, 1:] = xv[:, :, :-1]
    xsh[2].reshape(D, B, T)[:, :, :-1] = xv[:, :, 1:]
    xcat = np.concatenate([xsh.reshape(3 * D, BT),
                           np.ones((1, BT), np.float32)], axis=0)  # [241,256]
    w2 = np.concatenate([W[:, :, 0].T, W[:, :, 1].T, W[:, :, 2].T,
                         b[None, :]], axis=0)                      # [241,222]
    spk = np.zeros((128, 1468), np.float16)
    spk[0:121, 0:BT] = xcat[0:121].astype(np.float16)
    spk[0:120, BT:2 * BT] = xcat[121:241].astype(np.float16)
    spk[0:121, 2 * BT:2 * BT + CCEP] = w2[0:121].astype(np.float16)
    spk[0:120, 2 * BT + CCEP:2 * BT + 2 * CCEP] = w2[121:241].astype(np.float16)
    spk[:, 956:1468] = cp1

    zpad = np.concatenate(
        [np.zeros((B, HOP), np.float32), z[:, 0, :]], axis=1)     # [2, 33024]
    frames = np.lib.stride_tricks.sliding_window_view(
        zpad, WIN, axis=1)[:, ::HOP][:, :T]                       # [B, T, WIN]
    frp = frames.transpose(2, 0, 1).reshape(4, 128, B, T) \
        .transpose(1, 0, 2, 3).reshape(128, 4 * BT)               # [128, 1024]

    return spk, np.ascontiguousarray(frp).astype(np.float16)


def kernel(x, z, W, b):
    global LAST_RESULT
    in_maps = []
    frp = None
    for c in range(NCORES):
        cst = _CONSTS[c]
        if frp is None:
            spk0, frp = _prep_inputs(x, z, W, b, cst["cp1"])
            spk = spk0
        else:
            spk = spk0.copy()
            spk[:, 956:1468] = cst["cp1"]
        in_maps.append({"spk": spk, "frp": frp,
                        "zczs": cst["zczs"], "cp3": cst["cp3"]})

    nc = _get_nc()
    res = run_bass_kernel_spmd(nc, in_maps, list(range(NCORES)), trace=TRACE)
    LAST_RESULT = res
    out = np.zeros((B, 1, T * HOP), dtype=np.float32)
    for r in res.results:
        out += np.asarray(r["out"], dtype=np.float32)
    return out


# revision 23
# speedup vs baseline: 1.0257x; 1.0257x over previous
"""Trainium2 Bass kernel for nn_CCepLTVFilter.

Pipeline (per core; frequency-sharded across 8 cores, FS=128 freqs each):
  1. conv1d(x, W) + b            -> ccep[o, bt]          (PE, fp16 1-pass)
  2. Yr/Yi = DFT of padded ccep  -> [f, bt]              (PE, lhsT = CF/SF)
  3. mag = 10^(Yr/10) via tanh identity; paired sin/cos via two
     [128,512] ACT Sin ops: cs = [cos|sin], ns = [-sin|cos]
     (sin(x+pi) = -sin(x); single preloaded ACT table #18)
  4. Zr/Zi = 1025-pt DFT of z frames (host-transposed)   (PE, fp16)
  5. P = rn (cos + i sin)(zr + i zi) as three wide DVE ops:
     avdv = cs*[zr|zr], bvcv = ns*[zi|zi], u1u2 = avdv + bvcv,
     P = rn2 * u1u2 written straight into the guard-column layout
  6. OLA fused into the final DFT: out_b[t, 0:HOP] =
     Pr_b^T CO_l + Prs_b^T CO_r + Pi_b^T SO_l + Pis_b^T SO_r (PE)
     where the t-1 circular shift comes from 1-col guard columns in the
     packed P tile [128, 520] = [g|Pr_b0|g|Pr_b1|pad|g|Pi_b0|g|Pi_b1].

All matmul inputs are fp16 (m10 keeps rel err ~7e-3 vs the 2e-2 gate;
fp16 halves both DMA bytes and PE passes). CO/SO carry the Hann window
and a x16 rescale (fp16-normal range); 1/(16*1025) is folded into rn.
Inputs are packed into wide-row DMAs (>=2KB/partition) so HBM
transfers run near peak. Per-core outputs are partial sums of the full
[2,1,32768] output; summed on the host.
"""

import numpy as np

import concourse.bass as bass
import concourse.bacc as bacc
import concourse.mybir as mybir
import concourse.tile as tile
from concourse.bass_utils import run_bass_kernel_spmd
from concourse.instruction_name_ordered_set import InstructionNameOrderedSet

# ---------------- problem dims (hardcoded) ----------------
B, T, D = 2, 128, 80
CCEP = 222
FFT = 1024
HOP = 256
WIN = 2 * HOP            # 512
PAD = (FFT - CCEP) // 2  # 401
M = FFT + 1              # 1025-point transforms
BT = B * T               # 256
NCORES = 8
FS = FFT // NCORES       # 128 frequencies per core
OC = CCEP // 2           # 111 (o-chunk)
LAM = float(np.log(10.0) / 10.0)
COSC = 16.0              # CO/SO rescale; 1/(COSC*M) folded into rn

F32 = mybir.dt.float32
F16 = mybir.dt.float16
PI = float(np.pi)

ACT_TABLE_SIN_TANH = 18  # silu_and_others: covers Copy, Tanh, Sin

TRACE = False            # set by test harness for profiling
LAST_RESULT = None       # BassKernelResults of last run (for test harness)


# ---------------- host-side constants (input independent) ----------------
def _make_constants():
    o = np.arange(CCEP, dtype=np.float64)[:, None]
    f = np.arange(FFT, dtype=np.float64)[None, :]
    qn_idx = np.arange(1, CCEP // 2 + 1, dtype=np.float64)
    qnorm = np.concatenate([qn_idx[::-1], qn_idx])
    ang = 2.0 * np.pi * f * (o + PAD) / FFT
    CF = np.cos(ang) * (LAM / 2.0) / qnorm[:, None]      # [222,1024]
    SF = -np.sin(ang) / qnorm[:, None]

    u = np.arange(WIN, dtype=np.float64)[:, None]
    phi = 2.0 * np.pi * f * (u + FFT // 2) / M
    ZC = np.cos(phi)                                     # [512,1024]
    ZS = np.sin(phi)

    w = np.arange(WIN, dtype=np.float64)[None, :]
    th = 2.0 * np.pi * np.arange(FFT, dtype=np.float64)[:, None] * w / M
    win = 0.5 * (1.0 - np.cos(2.0 * np.pi * np.arange(WIN) / WIN))
    CO = np.cos(th) * win[None, :] * COSC                # [1024,512]
    SO = np.sin(th) * win[None, :] * COSC

    consts = []
    for c in range(NCORES):
        sl = slice(c * FS, (c + 1) * FS)
        # cf/sf [111, 256] each, cols (oc, f) -> packed into spk on host
        cfp = CF[:, sl].reshape(2, OC, FS).transpose(1, 0, 2).reshape(OC, 2 * FS)
        sfp = SF[:, sl].reshape(2, OC, FS).transpose(1, 0, 2).reshape(OC, 2 * FS)
        cp1 = np.zeros((128, 4 * FS), np.float32)
        cp1[0:OC, 0:2 * FS] = cfp
        cp1[0:OC, 2 * FS:4 * FS] = sfp
        # zczs [128, 1024]: zc(u-chunk mc, f) | zs
        zcp = ZC[:, sl].reshape(4, 128, FS).transpose(1, 0, 2).reshape(128, 4 * FS)
        zsp = ZS[:, sl].reshape(4, 128, FS).transpose(1, 0, 2).reshape(128, 4 * FS)
        zczs = np.concatenate([zcp, zsp], axis=1)
        # cp3 [128, 1024] = [co_l | co_r | so_l | so_r]
        cob, sob = CO[sl, :], SO[sl, :]
        cp3 = np.concatenate(
            [cob[:, :HOP], cob[:, HOP:], sob[:, :HOP], sob[:, HOP:]], axis=1)
        consts.append(dict(cp1=cp1.astype(np.float16),
                           zczs=zczs.astype(np.float16),
                           cp3=cp3.astype(np.float16)))
    return consts


_CONSTS = _make_constants()
_NC = None


# ---------------- device program ----------------
def _build_nc():
    nc = bacc.Bacc()
    # spk = spack (xcat+w2): 1912B rows, first-needed, alone on sync queue.
    # frq = frp (frames^T, 1024 cols) | cp1 (cf+sf, 512 cols): 3KB rows.
    sp_e = nc.dram_tensor("spk", [128, 956], F16, kind="ExternalInput")
    fr_e = nc.dram_tensor("frq", [128, 4 * BT + 4 * FS], F16,
                          kind="ExternalInput")
    zz_e = nc.dram_tensor("zczs", [128, 8 * FS], F16, kind="ExternalInput")
    c3_e = nc.dram_tensor("cp3", [128, 8 * FS], F16, kind="ExternalInput")
    out_e = nc.dram_tensor("out", [B, 1, T * HOP], F32, kind="ExternalOutput")

    with tile.TileContext(nc) as tc:
        with tc.tile_pool(name="sb", bufs=1) as sb, \
             tc.tile_pool(name="ps", bufs=1, space="PSUM") as ps:

            # ---- input DMAs; queue = issuing engine ----
            spk = sb.tile([128, 956], F16, tag="spk", name="spk")
            nc.sync.dma_start(out=spk[:], in_=sp_e[:, :])
            frq = sb.tile([128, 4 * BT + 4 * FS], F16, tag="frq", name="frq")
            nc.scalar.dma_start(out=frq[:], in_=fr_e[:, :])
            frp = frq[:, 0:4 * BT]
            zczs = sb.tile([128, 8 * FS], F16, tag="zczs", name="zczs")
            nc.gpsimd.dma_start(out=zczs[:], in_=zz_e[:, :])
            cp3 = sb.tile([128, 8 * FS], F16, tag="cp3", name="cp3")
            nc.gpsimd.dma_start(out=cp3[:], in_=c3_e[:, :])

            xcatA = spk[0:121, 0:BT]
            xcatB = spk[0:120, BT:2 * BT]
            w2A = spk[0:121, 2 * BT:2 * BT + CCEP]
            w2B = spk[0:120, 2 * BT + CCEP:2 * BT + 2 * CCEP]
            cf = frq[0:OC, 4 * BT:4 * BT + 2 * FS]
            sf = frq[0:OC, 4 * BT + 2 * FS:4 * BT + 4 * FS]
            zc = zczs[:, 0:4 * FS]
            zs = zczs[:, 4 * FS:8 * FS]
            co_l = cp3[:, 0:HOP]
            co_r = cp3[:, HOP:2 * HOP]
            so_l = cp3[:, 2 * HOP:3 * HOP]
            so_r = cp3[:, 3 * HOP:4 * HOP]

            # ---- conv: ccep[o, bt] = W2.T @ xcat (bias via ones row) ----
            ccep = []
            for c in range(2):
                pc = ps.tile([OC, BT], F32, tag=f"conv{c}", name=f"conv{c}")
                nc.tensor.matmul(pc[:, :], w2A[:, c * OC:(c + 1) * OC],
                                 xcatA, start=True, stop=False)
                nc.tensor.matmul(pc[:, :], w2B[:, c * OC:(c + 1) * OC],
                                 xcatB, start=False, stop=True)
                cs_ = sb.tile([OC, BT], F16, tag=f"ccep{c}", name=f"ccep{c}")
                # half on scalar, half on vector: halves the copy latency
                nc.scalar.copy(cs_[:, 0:T], pc[:, 0:T])
                nc.vector.tensor_copy(cs_[:, T:BT], pc[:, T:BT])
                ccep.append(cs_)

            # ---- step2: Yr/Yi [f_local, bt] ----
            yr = ps.tile([FS, BT], F32, tag="yr", name="yr")
            yi = ps.tile([FS, BT], F32, tag="yi", name="yi")
            for c in range(2):
                nc.tensor.matmul(yr[:, :], cf[:, c * FS:(c + 1) * FS],
                                 ccep[c][:, :], start=(c == 0), stop=(c == 1))
            for c in range(2):
                yi_mm = nc.tensor.matmul(yi[:, :], sf[:, c * FS:(c + 1) * FS],
                                         ccep[c][:, :],
                                         start=(c == 0), stop=(c == 1))

            # ---- step3 ----
            # ACT: tanh (feeds den/num), then three Sin ops building
            # csns [128, 1024] = [cos | sin | -sin | cos]
            th = sb.tile([FS, BT], F32, tag="th", name="th")
            nc.scalar.activation(th[:, :], yr[:, :],
                                 mybir.ActivationFunctionType.Tanh)
            # wr3 = [yic | yiw | yiwn]: wrapped yi+pi/2, yi, yi+pi
            wr3 = sb.tile([FS, 3 * BT], F32, tag="wr3", name="wr3")
            nc.vector.add_range_wrap(wr3[:, 0:BT], yi[:, :],
                                     PI / 2.0, PI, 2.0 * PI)
            nc.vector.add_range_wrap(wr3[:, BT:2 * BT], yi[:, :],
                                     0.0, PI, 2.0 * PI)
            nc.vector.add_range_wrap(wr3[:, 2 * BT:3 * BT], yi[:, :],
                                     PI, PI, 2.0 * PI)
            # csns [128, 768] = [cos | sin | -sin]; ns is a strided view
            # [-sin | cos] (cols 512:768 then 0:256)
            csns = sb.tile([FS, 3 * BT], F16, tag="csns", name="csns")
            for g in range(3):
                nc.scalar.activation(csns[:, g * BT:(g + 1) * BT],
                                     wr3[:, g * BT:(g + 1) * BT],
                                     mybir.ActivationFunctionType.Sin)
            cs = csns[:, 0:2 * BT]       # [cos | sin]
            csf = csns[:, :]
            ns = bass.AP(csf.tensor, csf.offset + 2 * BT,
                         [csf.ap[0], [-2 * BT, 2], [1, BT]])
            # rn = (1+t)/(1-t) / (M*COSC) ; den/num/rn on GPSIMD frees DVE
            den = sb.tile([FS, BT], F32, tag="den", name="den")
            nc.gpsimd.tensor_scalar(den[:, :], th[:, :], -1.0, 1.0,
                                    mybir.AluOpType.mult, mybir.AluOpType.add)
            s = 1.0 / (M * COSC)
            num = sb.tile([FS, BT], F32, tag="num", name="num")
            nc.gpsimd.tensor_scalar(num[:, :], th[:, :], s, s,
                                    mybir.AluOpType.mult, mybir.AluOpType.add)
            rcp = sb.tile([FS, BT], F32, tag="rcp", name="rcp")
            nc.vector.reciprocal_approx_fast(rcp[:, :], den[:, :])

            # ---- step4: Zr/Zi [f_local, bt] ----
            zr = ps.tile([FS, BT], F32, tag="zr", name="zr")
            zi = ps.tile([FS, BT], F32, tag="zi", name="zi")
            for mc in range(4):
                zr_mm = nc.tensor.matmul(zr[:, :], zc[:, mc * FS:(mc + 1) * FS],
                                         frp[:, mc * BT:(mc + 1) * BT],
                                         start=(mc == 0), stop=(mc == 3))
                if mc == 0:
                    # ordering-only dep: keep the PE queue in conv -> yr/yi
                    # -> zr/zi order (the scheduler's DMA model would
                    # otherwise front-run zr and head-of-line-block conv)
                    s_ = InstructionNameOrderedSet()
                    s_.add(yi_mm.ins.name)
                    zr_mm.ins.add_nosync_dependencies_from(s_)
            for mc in range(4):
                zi_mm = nc.tensor.matmul(zi[:, :], zs[:, mc * FS:(mc + 1) * FS],
                                         frp[:, mc * BT:(mc + 1) * BT],
                                         start=(mc == 0), stop=(mc == 3))
                if mc == 0:
                    s_ = InstructionNameOrderedSet()
                    s_.add(yi_mm.ins.name)
                    zi_mm.ins.add_nosync_dependencies_from(s_)

            # ---- step5 (DVE, wide fused ops) ----
            def rep2(pt):
                full = pt[:, :]
                return bass.AP(full.tensor, full.offset,
                               [full.ap[0], [0, 2], [1, BT]])

            avdv = sb.tile([FS, 2 * BT], F16, tag="avdv", name="avdv")
            nc.vector.tensor_tensor(avdv[:, :], cs[:, :], rep2(zr),
                                    mybir.AluOpType.mult)
            bvcv = sb.tile([FS, 2 * BT], F16, tag="bvcv", name="bvcv")
            nc.vector.tensor_tensor(bvcv[:, :], ns, rep2(zi),
                                    mybir.AluOpType.mult)
            u1u2 = sb.tile([FS, 2 * BT], F16, tag="u1u2", name="u1u2")
            nc.vector.tensor_tensor(u1u2[:, :], avdv[:, :], bvcv[:, :],
                                    mybir.AluOpType.add)
            rn = sb.tile([FS, BT], F16, tag="rn", name="rn")
            nc.gpsimd.tensor_tensor(rn[:, :], num[:, :], rcp[:, :],
                                    mybir.AluOpType.mult)

            # pq [128, 520] = [g|Pr_b0|g|Pr_b1|pad2 | g|Pi_b0|g|Pi_b1|pad2]
            pq = sb.tile([FS, 520], F16, tag="pq", name="pq")
            pqf = pq[:, :]
            pdst = bass.AP(pqf.tensor, pqf.offset + 1,
                           [pqf.ap[0], [260, 2], [129, 2], [1, T]])
            rnf = rn[:, :]
            rn4 = bass.AP(rnf.tensor, rnf.offset,
                          [rnf.ap[0], [0, 2], [T, 2], [1, T]])
            uf = u1u2[:, :]
            u4 = bass.AP(uf.tensor, uf.offset,
                         [uf.ap[0], [2 * T, 2], [T, 2], [1, T]])
            nc.vector.tensor_tensor(pdst, rn4, u4, mybir.AluOpType.mult)
            wdst = bass.AP(pqf.tensor, pqf.offset,
                           [pqf.ap[0], [260, 2], [129, 2], [1, 1]])
            wsrc = bass.AP(pqf.tensor, pqf.offset + T,
                           [pqf.ap[0], [260, 2], [129, 2], [1, 1]])
            nc.scalar.copy(wdst, wsrc)

            # ---- step6 + OLA (fused): per b, out_ob[t, j] =
            #  Pr_b^T co_l + Prs_b^T co_r + Pi_b^T so_l + Pis_b^T so_r
            for bb in range(B):
                opr = 1 + bb * 129
                opi = 261 + bb * 129
                pr_b = pq[:, opr:opr + T]
                prs_b = pq[:, opr - 1:opr - 1 + T]
                pi_b = pq[:, opi:opi + T]
                pis_b = pq[:, opi - 1:opi - 1 + T]
                ob = ps.tile([T, HOP], F32, tag=f"ob{bb}", name=f"ob{bb}")
                # unshifted terms first: the guard-col wrap copy lands
                # while they run
                nc.tensor.matmul(ob[:, :], pr_b, co_l, start=True, stop=False)
                nc.tensor.matmul(ob[:, :], pi_b, so_l, start=False, stop=False)
                nc.tensor.matmul(ob[:, :], prs_b, co_r, start=False, stop=False)
                nc.tensor.matmul(ob[:, :], pis_b, so_r, start=False, stop=True)
                obs = sb.tile([T, HOP], F32, tag=f"obs{bb}", name=f"obs{bb}")
                if bb == 0:
                    nc.scalar.copy(obs[:, :], ob[:, :])
                else:
                    # halve the tail latency: copy halves on both engines
                    nc.vector.tensor_copy(obs[:, 0:T], ob[:, 0:T])
                    nc.scalar.copy(obs[:, T:HOP], ob[:, T:HOP])
                dst = bass.AP(out_e[:, :, :].tensor, bb * T * HOP,
                              [[HOP, T], [1, HOP]])
                eng = nc.sync if bb == 0 else nc.scalar
                eng.dma_start(out=dst, in_=obs[:, :])

    return nc


def _patch_act_table(nc):
    """Pre-place a single ACT table load (table 18 covers Copy+Tanh+Sin)
    instead of the default pass's two loads (exp table then trig table).
    Placed just before the first InstActivation so the scalar engine's
    DMA issues at the top of the block are not delayed by the load."""
    def my_insert():
        for b in nc.main_func.blocks:
            idx = None
            for j, i in enumerate(b.instructions):
                if isinstance(i, mybir.InstActivation):
                    idx = j
                    break
            if idx is None:
                continue
            ld = mybir.InstLoadActFuncSet(
                name=nc.get_next_instruction_name(),
                act_func_set_id=ACT_TABLE_SIN_TANH, ins=[], outs=[])
            ld.engine = mybir.EngineType.Activation
            nc.register_instruction(ld)
            b.instructions.insert(idx, ld)
            return
    nc.insert_act_table_loads = my_insert


def _get_nc():
    global _NC
    if _NC is None:
        _NC = _build_nc()
        _patch_act_table(_NC)
        _NC.finalize()
    return _NC


# ---------------- host orchestration ----------------
def _prep_inputs(x, z, W, b):
    x = np.ascontiguousarray(np.asarray(x, dtype=np.float32))
    z = np.ascontiguousarray(np.asarray(z, dtype=np.float32))
    W = np.ascontiguousarray(np.asarray(W, dtype=np.float32))
    b = np.ascontiguousarray(np.asarray(b, dtype=np.float32))

    xT = np.ascontiguousarray(x.reshape(BT, D).T)                 # [80, 256]
    xsh = np.zeros((3, D, BT), np.float32)
    xsh[1] = xT
    xv = xT.reshape(D, B, T)
    xsh[0].reshape(D, B, T)[:, :, 1:] = xv[:, :, :-1]
    xsh[2].reshape(D, B, T)[:, :, :-1] = xv[:, :, 1:]
    xcat = np.concatenate([xsh.reshape(3 * D, BT),
                           np.ones((1, BT), np.float32)], axis=0)  # [241,256]
    w2 = np.concatenate([W[:, :, 0].T, W[:, :, 1].T, W[:, :, 2].T,
                         b[None, :]], axis=0)                      # [241,222]
    spk = np.zeros((128, 956), np.float16)
    spk[0:121, 0:BT] = xcat[0:121].astype(np.float16)
    spk[0:120, BT:2 * BT] = xcat[121:241].astype(np.float16)
    spk[0:121, 2 * BT:2 * BT + CCEP] = w2[0:121].astype(np.float16)
    spk[0:120, 2 * BT + CCEP:2 * BT + 2 * CCEP] = w2[121:241].astype(np.float16)

    zpad = np.concatenate(
        [np.zeros((B, HOP), np.float32), z[:, 0, :]], axis=1)     # [2, 33024]
    frames = np.lib.stride_tricks.sliding_window_view(
        zpad, WIN, axis=1)[:, ::HOP][:, :T]                       # [B, T, WIN]
    frp = frames.transpose(2, 0, 1).reshape(4, 128, B, T) \
        .transpose(1, 0, 2, 3).reshape(128, 4 * BT)               # [128, 1024]

    return spk, np.ascontiguousarray(frp).astype(np.float16)


def _pack_frq(frp, cp1):
    frq = np.empty((128, 4 * BT + 4 * FS), np.float16)
    frq[:, 0:4 * BT] = frp
    frq[:, 4 * BT:] = cp1
    return frq


def kernel(x, z, W, b):
    global LAST_RESULT
    spk, frp = _prep_inputs(x, z, W, b)
    in_maps = []
    for c in range(NCORES):
        cst = _CONSTS[c]
        in_maps.append({"spk": spk, "frq": _pack_frq(frp, cst["cp1"]),
                        "zczs": cst["zczs"], "cp3": cst["cp3"]})

    nc = _get_nc()
    res = run_bass_kernel_spmd(nc, in_maps, list(range(NCORES)), trace=TRACE)
    LAST_RESULT = res
    out = np.zeros((B, 1, T * HOP), dtype=np.float32)
    for r in res.results:
        out += np.asarray(r["out"], dtype=np.float32)
    return out
